# revision 24
# baseline (speedup 1.0000x reference)
"""Bidirectional 2-layer LSTM (with replicated hf1-input bug) + per-step linear,
as a Trainium2 Bass/Tile kernel, data-parallel over batch across 8 NeuronCores.

2-stream software-pipelined design (v2):
  - Per core B_loc=256 batch, split into 2 streams of W=128 (stream r = batch
    half). The two streams run the same wave schedule offset by half a wave,
    so stream B's tensor leg overlaps stream A's vector/scalar leg. This
    converts the latency-bound engine ping-pong of the single-stream design
    into a throughput-bound pipeline.
  - All recurrent state (h, x, weights, sigma/tanh outputs, c) is fp16:
    PE matmuls run 1 cycle/col (vs 4 for fp32 LOW_HIGH), DVE ops get 2x/4x
    perf modes. PSUM stays fp32. Validated: rel err ~6e-3 vs fp32 reference.
  - Packed state tile pkq [128 rows, 4 slots, 2*W] (slot = wave%4):
    rows 0:32 hf0, 32:64 hf1, 64:96 hb0, 96(x)/97(xb)/98(ones), 99:128 zero.
    4 matmuls per wave per stream, one per GATE, K=128 x M=128 x N=W.
    Stationary M-columns pack all four cells [f0,f1,b0,b1] (32 cols each);
    bias enters via the ones row; g-gate weights pre-scaled by 2 so a merged
    Sigmoid yields sigma(2g), tanh(g) = 2*sigma(2g)-1.
  - Software-skewed cells: wave w computes f0@w, f1@w-1, b0@w-1, b1@w-2.
  - Per half-step (r, w) engine bundles (queue order chosen to minimize
    stalls): TENSOR [outproj2(r,w-1), gates(r,w) x4, outproj1(r,w)];
    SCALAR [tanh(prev), sigmoid(r,w)]; VECTOR [h2a(prev), copies, TS, u, add];
    GPSIMD [h2b(prev), v].  outproj2 is deferred one wave so it never stalls
    the tensor queue on the gpsimd h2b of the same wave.
  - gps PSUM [128, 2, 4W] per parity: stream halves on separate banks so the
    PSUM bank tracker never falsely serializes the streams.
  - Output: per-wave [1,W] out rows accumulate in PSUM (wout1 over pkq with
    bias+hf1, wout2 over hb1); evacuated via DVE every 4 waves per stream to
    a DRAM scratch in [t, b] layout; transposed to [b, t] with PE transposes.
"""

import sys

sys.path.insert(0, "/opt/trn_rl_repo")

import numpy as np
import concourse.bass as bass
import concourse.tile as tile
import concourse.mybir as mybir
import bass_rust
from concourse.bass_utils import run_bass_kernel_spmd

S, B, H = 1024, 2048, 32
NCORES = 8
BL = B // NCORES  # 256
W = BL // 2  # 128 per stream

F32 = mybir.dt.float32
BF = mybir.dt.float16  # 16-bit compute dtype (fp16: 10-bit mantissa)
AF = mybir.ActivationFunctionType
OP = mybir.AluOpType

NP_BF16 = np.float16  # numpy counterpart of BF

# cell order along M-columns: [f0: 0:32, f1: 32:64, b0: 64:96, b1: 96:128]
CELL_COL = {"f0": 0, "f1": 32, "b0": 64, "b1": 96}
ROW_HF0, ROW_HF1, ROW_HB0 = 0, 32, 64
ROW_X, ROW_XB, ROW_ONES = 96, 97, 98


def _split_excess_waits(nc, max_waits=1):
    """walrus codegen supports only one sync-wait per instruction; split
    extras onto inserted wait-only drains."""
    n = 0
    for f in nc.m.functions:
        for bb in f.blocks:
            newl = []
            dirty = False
            for ins in bb.instructions:
                si = ins.sync_info
                waits = list(si.on_wait) if si is not None else []
                if len(waits) > max_waits:
                    dirty = True
                    k = len(waits) - max_waits
                    i = 0
                    while i < k:
                        chunk = waits[i : min(i + max_waits, k)]
                        d = mybir.InstDrain(name=f"zwsplit-{n}", is_reset_sema=False)
                        n += 1
                        d.engine = ins.engine
                        d.sync_info = bass_rust.SyncInfo(on_wait=chunk, on_update=[])
                        newl.append(d)
                        i += max_waits
                    si.on_wait = waits[k:]
                    ins.sync_info = si
                newl.append(ins)
            if dirty:
                bb.instructions = newl
    return n


def _gate_block(Wmat, gi):
    return Wmat[gi * H : (gi + 1) * H]


def build_weights(Wih_f0, Whh_f0, b_f0, Wih_f1, Whh_f1, b_f1,
                  Wih_b0, Whh_b0, b_b0, Wih_b1, Whh_b1, b_b1, Wlin, blin):
    Wg = np.zeros((4, 128, 128), np.float32)
    for gi in range(4):
        sc = 2.0 if gi == 2 else 1.0  # tanh-gate pre-scale
        c = CELL_COL["f0"]
        Wg[gi, ROW_X, c : c + H] = _gate_block(Wih_f0, gi)[:, 0] * sc
        Wg[gi, ROW_ONES, c : c + H] = _gate_block(b_f0, gi) * sc
        Wg[gi, ROW_HF0 : ROW_HF0 + H, c : c + H] = _gate_block(Whh_f0, gi).T * sc
        c = CELL_COL["f1"]
        Wg[gi, ROW_ONES, c : c + H] = _gate_block(b_f1, gi) * sc
        Wg[gi, ROW_HF0 : ROW_HF0 + H, c : c + H] = _gate_block(Wih_f1, gi).T * sc
        Wg[gi, ROW_HF1 : ROW_HF1 + H, c : c + H] = _gate_block(Whh_f1, gi).T * sc
        c = CELL_COL["b0"]
        Wg[gi, ROW_XB, c : c + H] = _gate_block(Wih_b0, gi)[:, 0] * sc
        Wg[gi, ROW_ONES, c : c + H] = _gate_block(b_b0, gi) * sc
        Wg[gi, ROW_HB0 : ROW_HB0 + H, c : c + H] = _gate_block(Whh_b0, gi).T * sc
        c = CELL_COL["b1"]
        Wg[gi, ROW_ONES, c : c + H] = _gate_block(b_b1, gi) * sc
        Wg[gi, ROW_HB0 : ROW_HB0 + H, c : c + H] = _gate_block(Wih_b1, gi).T * sc
        Wg[gi, ROW_HF1 : ROW_HF1 + H, c : c + H] = _gate_block(Whh_b1, gi).T * sc

    wout1 = np.zeros((128, 1), np.float32)
    wout1[ROW_ONES, 0] = blin[0]
    wout1[ROW_HF1 : ROW_HF1 + H, 0] = Wlin[0, 0:H]
    wout2 = np.zeros((128, 1), np.float32)
    wout2[96:128, 0] = Wlin[0, H : 2 * H]
    return Wg.astype(NP_BF16), wout1.astype(NP_BF16), wout2.astype(NP_BF16)


def build_xpair(x_shard, s):
    """xp[p, row, d, :]: row for packed partition (0=x, 1=xb, 2=ones) of
    wave w = 2p + d. Batch axis is [stream r, W] flattened."""
    bl = x_shard.shape[1]
    npair = s // 2 + 1
    xp = np.zeros((npair, 3, 2, bl), np.float32)
    xp[:, 2, :, :] = 1.0
    for p in range(npair):
        for d in range(2):
            w = 2 * p + d
            if w < s:
                xp[p, 0, d] = x_shard[w]
            if w >= 1:
                xp[p, 1, d] = x_shard[(s + 1 - w) % s]
    return xp.astype(NP_BF16)


N_WARM = 2  # wide dependency-free PE-warming matmuls per half-step


def build_nc(s=S, unroll_pairs=8, dbg=False):
    assert s % 16 == 0 and s >= 32, "schedule requires s = 16k"
    npair = s // 2 + 1
    nc = bass.Bass("TRN2", target_bir_lowering=False, debug=False,
                   num_devices=NCORES)

    xd = nc.declare_dram_parameter("xpair", [npair, 3, 2, BL], BF, isOutput=False)
    wgd = nc.declare_dram_parameter("Wg", [128, 4, 128], BF, isOutput=False)
    wo1d = nc.declare_dram_parameter("wout1", [128, 1], BF, isOutput=False)
    wo2d = nc.declare_dram_parameter("wout2", [128, 1], BF, isOutput=False)
    idd = nc.declare_dram_parameter("ident", [128, 128], F32, isOutput=False)
    outd = nc.declare_dram_parameter("out", [BL, s], F32, isOutput=True)
    oscr = nc.dram_tensor("oscr", [s // 2, 2, BL], F32)

    with tile.TileContext(nc) as tc:
        with (
            tc.tile_pool(name="const", bufs=1) as cpool,
            tc.tile_pool(name="state", bufs=1) as spool,
            tc.tile_pool(name="psum", bufs=1, space="PSUM") as ppool,
        ):
            wg_t = cpool.tile([128, 4, 128], BF)
            wo1_t = cpool.tile([128, 1], BF)
            wo2_t = cpool.tile([128, 1], BF)
            id_t = cpool.tile([128, 128], F32)
            nc.sync.dma_start(wg_t[:], wgd[:])
            nc.sync.dma_start(wo1_t[:], wo1d[:])
            nc.sync.dma_start(wo2_t[:], wo2d[:])
            nc.sync.dma_start(id_t[:], idd[:])

            pkq = spool.tile([128, 4, BL], BF, name="pkq")
            c_all = spool.tile([128, BL], BF, name="c_all")
            # sig/gps layout: [128, r, gate*W]; the two r-halves of gps land
            # on separate PSUM banks so the bank tracker never falsely
            # serializes the streams.
            sig = [spool.tile([128, 2, 4 * W], BF, name=f"sig{j}") for j in range(2)]
            wt = [spool.tile([128, BL], BF, name=f"wt{j}") for j in range(2)]
            u = [spool.tile([128, BL], BF, name=f"u{j}") for j in range(2)]
            v = [spool.tile([128, BL], BF, name=f"v{j}") for j in range(2)]
            tct = [spool.tile([128, BL], BF, name=f"tct{j}") for j in range(2)]
            hb1 = [spool.tile([128, BL], BF, name=f"hb1{j}") for j in range(4)]
            # osb layout [1, slot, r, W] = t-major rows matching oscr, so the
            # group DMA is a contiguous 2-row write with a dynamic offset.
            osb = [spool.tile([1, 4, 2, W], F32, name=f"osb{j}") for j in range(2)]
            warm = spool.tile([128, 4 * W], BF, name="warm")

            gps = [ppool.tile([128, 2, 4 * W], F32, name=f"gps{j}") for j in range(2)]
            ops_ = [ppool.tile([1, 2, 4, W], F32, name=f"ops{j}") for j in range(2)]

            # ---- init ----
            nc.vector.memset(warm[:], 0.0)
            nc.vector.memset(pkq[:], 0.0)
            nc.vector.memset(c_all[:], 0.0)
            for j in range(4):
                nc.vector.memset(hb1[j][:], 0.0)

            def rs(t_ap, r):
                """stream-r column half of a [.., BL] AP"""
                return t_ap[:, r * W : (r + 1) * W]

            def hs(r, wm, prev, do2, do1, dma_src=None, copy=None,
                   skip_h2a_prev=False):
                """One half-step: stream r at wave w (wm = w mod 16).

                prev = (pr, pwm): the immediately preceding half-step, whose
                tanh/h2a/h2b are emitted here. copy = (par, rows_ap): emit
                the out-group evacuation for stream r.
                """
                j = wm % 4
                bb = wm % 2
                gp = bb if r == 0 else 1 - bb
                mv = pkq[:, j, r * W : (r + 1) * W]  # moving tile [128, W]
                if dma_src is not None:
                    nc.sync.dma_start(pkq[96:99, j : j + 2, :], dma_src)
                # ---- TENSOR ----
                for g in range(4):
                    nc.tensor.matmul(
                        gps[gp][:, r, g * W : (g + 1) * W],
                        wg_t[:, g, :],
                        mv,
                        start=True, stop=True,
                    )
                if do2:
                    pwm2 = (wm - 1) % 16
                    o_sl = (pwm2 - 2) % 4
                    o_par = (((pwm2 - 2) % 16) // 4) % 2
                    nc.tensor.matmul(
                        ops_[o_par][0:1, r, o_sl, :],
                        wo2_t[:],
                        rs(hb1[pwm2 % 4][:], r),
                        start=False, stop=True,
                    )
                if do1:
                    o_sl = (wm - 2) % 4
                    o_par = (((wm - 2) % 16) // 4) % 2
                    nc.tensor.matmul(
                        ops_[o_par][0:1, r, o_sl, :],
                        wo1_t[:],
                        mv,
                        start=True, stop=False,
                    )
                # ---- SCALAR ----
                if prev is not None:
                    pr, pwm = prev
                    nc.scalar.activation(
                        rs(tct[pwm % 2][:], pr), rs(c_all[:], pr), AF.Tanh
                    )
                nc.scalar.activation(
                    sig[gp][:, r, :], gps[gp][:, r, :], AF.Sigmoid
                )
                # ---- GPSIMD ---- (emitted before the vector `add` so the
                # `v` write precedes its read in program order)
                sg = sig[gp]
                nc.gpsimd.tensor_tensor(
                    rs(v[bb][:], r), sg[:, r, W : 2 * W], rs(c_all[:], r),
                    OP.mult,
                )
                if prev is not None:
                    pr, pwm = prev
                    pgp = (pwm % 2) if pr == 0 else 1 - (pwm % 2)
                    nc.gpsimd.tensor_tensor(
                        hb1[pwm % 4][96:128, pr * W : (pr + 1) * W],
                        sig[pgp][96:128, pr, 3 * W : 4 * W],
                        tct[pwm % 2][96:128, pr * W : (pr + 1) * W],
                        OP.mult,
                    )
                # ---- VECTOR ----
                if prev is not None and not skip_h2a_prev:
                    pr, pwm = prev
                    pgp = (pwm % 2) if pr == 0 else 1 - (pwm % 2)
                    with tc.high_priority(offset=40):
                        nc.vector.tensor_tensor(
                            pkq[0:96, (pwm + 1) % 4, pr * W : (pr + 1) * W],
                            sig[pgp][0:96, pr, 3 * W : 4 * W],
                            tct[pwm % 2][0:96, pr * W : (pr + 1) * W],
                            OP.mult,
                        )
                if copy is not None:
                    cpar, rows_ap = copy
                    nc.vector.tensor_copy(
                        osb[cpar][0:1, :, r, :], ops_[cpar][0:1, r, :, :]
                    )
                    if r == 1:  # both stream halves landed; flush the group
                        nc.sync.dma_start(rows_ap, osb[cpar][:])
                nc.vector.scalar_tensor_tensor(
                    rs(u[bb][:], r), sg[:, r, 2 * W : 3 * W], -0.5,
                    sg[:, r, 0:W], OP.add, OP.mult,
                )
                nc.vector.scalar_tensor_tensor(
                    rs(c_all[:], r), rs(u[bb][:], r), 2.0,
                    rs(v[bb][:], r), OP.mult, OP.add,
                )

            # ================= PROLOGUE: waves 0..17 =================
            prev = None
            for w in range(0, 18):
                for r in range(2):
                    dma_src = (xd[w // 2 : w // 2 + 1, :, :, :]
                               if (r == 0 and w % 2 == 0) else None)
                    copy = None
                    if w >= 8 and w % 4 == 0:
                        g = (w - 8) // 4
                        copy = (g % 2, oscr[2 * g : 2 * g + 2, :, :])
                    hs(r, w % 16, prev,
                       do2=(w - 1 >= 2), do1=(w >= 2),
                       dma_src=dma_src, copy=copy)
                    prev = (r, w % 16)
                    # junk-state cleanup: zeroing c rows before this stream's
                    # tanh runs makes the junk h2a/h2b outputs zero as well.
                    if w == 0:
                        for p0 in (32, 64, 96):
                            nc.vector.memset(
                                c_all[p0 : p0 + 32, r * W : (r + 1) * W], 0.0)
                    elif w == 1:
                        nc.vector.memset(c_all[96:128, r * W : (r + 1) * W], 0.0)

            # ============ MAIN LOOP: pairs 9 .. s//2, waves 18..s+1 ========
            with tc.For_i(9, s // 2 + 1, 8) as ip:
                for k in range(8):
                    for d in range(2):
                        wm = (2 + 2 * k + d) % 16  # w = 2(ip+k)+d mod 16
                        for r in range(2):
                            dma_src = (xd[bass.ds(ip + k, 1), :, :, :]
                                       if (r == 0 and d == 0) else None)
                            copy = None
                            if d == 0 and k % 2 == 1:
                                cpar = (((9 + k - 4) // 2) % 2)
                                copy = (cpar, oscr[bass.ds(ip + k - 4, 2), :, :])
                            hs(r, wm, prev, do2=True, do1=True,
                               dma_src=dma_src, copy=copy)
                            prev = (r, wm)

            # ================= EPILOGUE: wave s+1 trailing ops ============
            wl = (s + 1) % 16  # = 1
            # tanh/h2b for (B, s+1); h2a not needed (no wave s+2)
            nc.scalar.activation(rs(tct[wl % 2][:], 1), rs(c_all[:], 1), AF.Tanh)
            gpl = 1 - (wl % 2)  # gp for (r=1, wl)
            nc.gpsimd.tensor_tensor(
                hb1[(1 + s) % 4][96:128, W:BL],
                sig[gpl][96:128, 1, 3 * W : 4 * W],
                tct[wl % 2][96:128, W:BL],
                OP.mult,
            )
            for wep in (s + 1,):  # pending deferred outproj2
                wem = wep % 16
                o_sl = (wem - 2) % 4
                o_par = (((wem - 2) % 16) // 4) % 2
                for r in range(2):
                    nc.tensor.matmul(
                        ops_[o_par][0:1, r, o_sl, :],
                        wo2_t[:],
                        rs(hb1[wep % 4][:], r),
                        start=False, stop=True,
                    )
            gl = s // 4 - 1  # last out group (par 1)
            for r in range(2):
                nc.vector.tensor_copy(osb[1][0:1, :, r, :], ops_[1][0:1, r, :, :])
            nc.sync.dma_start(oscr[2 * gl : 2 * gl + 2, :, :], osb[1][:])

            # ======== END PHASE: transpose oscr [t, b] -> out [b, t] ======
            nchunk = s // 128
            outsb = spool.tile([128, 2, nchunk, 128], F32, name="outsb")
            stg = [spool.tile([128, BL], F32, name=f"stg{j}") for j in range(2)]
            for ch in range(nchunk):
                st = stg[ch % 2]
                nc.sync.dma_start(st[:], oscr[ch * 64 : (ch + 1) * 64, :, :])
                for g2 in range(2):
                    tp = gps[g2][:, 0, 0:128]
                    nc.tensor.transpose(tp, st[:, g2 * 128 : (g2 + 1) * 128], id_t[:])
                    nc.vector.tensor_copy(outsb[:, g2, ch, :], tp)
            nc.sync.dma_start(outd[0:128, :], outsb[:, 0, :, :])
            nc.sync.dma_start(outd[128:256, :], outsb[:, 1, :, :])

    _split_excess_waits(nc)
    return nc


_NC_CACHE = {}


def _get_nc(s=S, unroll_pairs=8, dbg=False):
    key = (s, unroll_pairs, dbg)
    if key not in _NC_CACHE:
        _NC_CACHE[key] = build_nc(s, unroll_pairs, dbg)
    return _NC_CACHE[key]


def run(x, weights, s=S, unroll_pairs=8, dbg=False, **rkw):
    """x: [s, B] fp32 (already squeezed); weights: dict of reference arrays."""
    Wg, wout1, wout2 = build_weights(**weights)
    nc = _get_nc(s, unroll_pairs, dbg)
    ident = np.eye(128, dtype=np.float32)
    in_maps = []
    for c in range(NCORES):
        xs = np.ascontiguousarray(x[:, c * BL : (c + 1) * BL])
        in_maps.append(
            {"xpair": build_xpair(xs, s),
             "Wg": np.ascontiguousarray(Wg.transpose(1, 0, 2)),
             "wout1": wout1, "wout2": wout2, "ident": ident}
        )
    res = run_bass_kernel_spmd(nc, in_maps, list(range(NCORES)), **rkw)
    out = np.concatenate([res.results[c]["out"] for c in range(NCORES)], axis=0)
    return out, res


def kernel(x, Wih_f0, Whh_f0, b_f0, Wih_f1, Whh_f1, b_f1,
           Wih_b0, Whh_b0, b_b0, Wih_b1, Whh_b1, b_b1, Wlin, blin, future):
    assert int(future) == 0, "kernel hardcodes future=0"
    x = np.asarray(x, np.float32)
    s, b, _ = x.shape
    assert (s, b) == (S, B)
    weights = dict(
        Wih_f0=np.asarray(Wih_f0, np.float32), Whh_f0=np.asarray(Whh_f0, np.float32),
        b_f0=np.asarray(b_f0, np.float32),
        Wih_f1=np.asarray(Wih_f1, np.float32), Whh_f1=np.asarray(Whh_f1, np.float32),
        b_f1=np.asarray(b_f1, np.float32),
        Wih_b0=np.asarray(Wih_b0, np.float32), Whh_b0=np.asarray(Whh_b0, np.float32),
        b_b0=np.asarray(b_b0, np.float32),
        Wih_b1=np.asarray(Wih_b1, np.float32), Whh_b1=np.asarray(Whh_b1, np.float32),
        b_b1=np.asarray(b_b1, np.float32),
        Wlin=np.asarray(Wlin, np.float32), blin=np.asarray(blin, np.float32),
    )
    out, _ = run(x[:, :, 0], weights, s=S)
    return out
